# revision 64
# baseline (speedup 1.0000x reference)
"""Multi-head attention TRN2 kernel, head-sharded across 8 NeuronCores.

Reference computation (fp32):
    qkv = x @ w_qkv + b_qkv            x:[4,2048,1024] w_qkv:[1024,3072]
    q,k,v per head (16 heads, d=64)
    out = softmax(q k^T / 8) v         per (batch, head)
    y = out @ w_out + b_out
Core c owns heads {2c, 2c+1} (tensor-parallel split of w_qkv columns /
w_out rows); host sums the 8 partial y's (+ b_out).

v3 design (~411us HW, measured 410.8-412.0 across runs):
  - Steady state is bound by the coupled PE-issue stream (~2.2us of
    matmuls per 2-kt group) and the scalar exp chain (2 ACTIVATEs x
    ~1.1us per group); both run ~90+% busy.
  - All projection / phase-C work is chopped into ~0.4-0.9us emission
    quanta, pumped two per 2-kt attention group between the attnV
    block and the score pairs, so the in-order PE queue always holds
    ~2.2us of work per 2 exps and neither engine starves.
  - Zero-padded score stationaries: kTa=[K_a|0], kTb=[0|K_b] make the
    score matmuls full (128,128)-tile-config matmuls like everything
    else.  Every PE tile-geometry switch costs ~100ns (LDWEIGHTS
    cannot overlap across a reconfiguration), and the previous
    row-tiled K=64 pairs paid that twice per kt slot.
  - Same-shape matmuls are batched per group (4 attnV, then 2x2
    score matmuls) for the same reason.
  - Scalar diet: exp is (almost) the only scalar-engine instruction;
    softmax denominators come free as attnV ones-columns, get
    reciprocal'd on the DVE from an SBUF-staged row, broadcast on
    GPSIMD, multiplied on DVE one qc behind attention.
  - Phase-C quanta are held back one qc so their matmuls never
    head-of-line-block the PE queue on the in-flight normalize chain.
  - A 60-matmul warm-up plus dense emission keeps the HAM governor at
    2.4GHz from t=10us; filler matmuls ride out the final normalize
    chain so the flush is not down-clocked; the last batch borrows
    the idle proj PSUM bank to double-buffer the tail's phase-C.
  - PSUM: s_ab 2x2 banks, o_a/o_b 2, proj accum 1, phase-C accum 1.
"""
import sys
import types
from collections import deque

import numpy as np

B, S, E, H, D = 4, 2048, 1024, 16, 64
TOK = B * S          # 8192 tokens
NCORE = 8
HPC = H // NCORE     # heads per core = 2
CH = 512             # token chunk (matmul moving dim)
NQC = S // CH        # 4 chunks per batch
KE = E // 128        # 8 contraction tiles for the projections
KT = S // 128        # 16 key tiles per batch
VW = 2 * (D + 1)     # 130: per key-tile V block [v_a | 1 | v_b | 1]

_CACHE = {}


def _install_ntff_hook():
    if "antenv.axon_hooks" in sys.modules:
        return
    try:
        import antenv
    except ImportError:
        return
    mod = types.ModuleType("antenv.axon_hooks")
    mod._hook = None

    def set_axon_ntff_profile_hook(h):
        mod._hook = h

    def get_axon_ntff_profile_hook():
        return mod._hook

    mod.set_axon_ntff_profile_hook = set_axon_ntff_profile_hook
    mod.get_axon_ntff_profile_hook = get_axon_ntff_profile_hook
    antenv.axon_hooks = mod
    sys.modules["antenv.axon_hooks"] = mod


def _build(with_qkv_bias: bool):
    import concourse.tile as tile
    from concourse import bacc, mybir

    f32 = mybir.dt.float32
    bf16 = mybir.dt.bfloat16
    EXP = mybir.ActivationFunctionType.Exp
    MULT = mybir.AluOpType.mult

    nc = bacc.Bacc("TRN2", target_bir_lowering=False, debug=False,
                   num_devices=NCORE)

    xT = nc.dram_tensor("xT", [E, TOK], bf16, kind="ExternalInput").ap()
    wq = nc.dram_tensor("wq", [E, 128], bf16, kind="ExternalInput").ap()
    wk = nc.dram_tensor("wk", [E, 128], bf16, kind="ExternalInput").ap()
    wv = nc.dram_tensor("wv", [E, 128], bf16, kind="ExternalInput").ap()
    wo = nc.dram_tensor("wo", [128, E], bf16, kind="ExternalInput").ap()
    ident = nc.dram_tensor("ident", [128, 128], bf16,
                           kind="ExternalInput").ap()
    if with_qkv_bias:
        bq = nc.dram_tensor("bq", [1, 128], bf16, kind="ExternalInput").ap()
        bk = nc.dram_tensor("bk", [1, 128], bf16, kind="ExternalInput").ap()
        bv = nc.dram_tensor("bv", [1, 128], bf16, kind="ExternalInput").ap()
    y = nc.dram_tensor("y", [TOK, E], bf16,
                       kind="ExternalOutput").ap()

    HC = CH // 2        # 256: half-chunk (half-bank accumulator width)

    with tile.TileContext(nc) as tc:
        with tc.tile_pool(name="res", bufs=1) as res, \
             tc.tile_pool(name="qp", bufs=2) as qp, \
             tc.tile_pool(name="kp", bufs=2) as kp, \
             tc.tile_pool(name="vp", bufs=2) as vp, \
             tc.tile_pool(name="xa", bufs=16) as xa, \
             tc.tile_pool(name="va", bufs=2) as va, \
             tc.tile_pool(name="eb", bufs=6) as eb, \
             tc.tile_pool(name="otp", bufs=2) as otp, \
             tc.tile_pool(name="onp", bufs=2) as onp, \
             tc.tile_pool(name="rcp", bufs=2) as rcp, \
             tc.tile_pool(name="yc", bufs=4) as yc, \
             tc.tile_pool(name="pa", bufs=1, space="PSUM") as pa, \
             tc.tile_pool(name="pc", bufs=1, space="PSUM") as pc, \
             tc.tile_pool(name="pbs", bufs=2, space="PSUM") as pbs, \
             tc.tile_pool(name="po", bufs=1, space="PSUM") as po:
            # --- residents ---
            wq_sb = res.tile([128, KE, 128], bf16)
            wk_sb = res.tile([128, KE, 128], bf16)
            wv_sb = res.tile([128, KE, 128], bf16)
            wo_sb = res.tile([128, E], bf16)
            id_sb = res.tile([128, 128], bf16)
            warm = res.tile([128, 256], bf16)

            wview = lambda w: w.rearrange("(k p) m -> p k m", p=128)
            nc.vector.memset(warm[:], 0.0)
            nc.sync.dma_start(wq_sb[:], wview(wq))
            nc.sync.dma_start(wk_sb[:], wview(wk))
            nc.sync.dma_start(wv_sb[:], wview(wv))

            if with_qkv_bias:
                ones_sb = res.tile([1, CH], bf16)
                nc.vector.memset(ones_sb[:], 1.0)
                bq_sb = res.tile([1, 128], bf16)
                bk_sb = res.tile([1, 128], bf16)
                bv_sb = res.tile([1, 128], bf16)
                nc.sync.dma_start(bq_sb[:], bq)
                nc.sync.dma_start(bk_sb[:], bk)
                nc.sync.dma_start(bv_sb[:], bv)

            def emit_warmup():
                # HAM warm-up: ~3us of filler matmuls so the first real
                # phase runs at 2.4GHz (borrows a phase-C PSUM slot).
                ps_w = pc.tile([128, 2, HC], f32, name="ps_y")
                for _ in range(60):
                    nc.tensor.matmul(ps_w[:, 0, :], warm[:, 0:128],
                                     warm[:, 0:256], start=True, stop=True)

            def load_late_weights():
                nc.sync.dma_start(id_sb[:], ident)
                nc.sync.dma_start(wo_sb[:], wo)

            def proj_quanta(b):
                """QKV projection for batch b as a list of emission quanta
                (~0.4-0.9us of PE work each).  Returns (qT, kT, vb, quanta);
                the tiles are allocated now, written when quanta run."""
                qT = qp.tile([128, NQC, CH], bf16, name="qT")
                # two zero-padded K tiles: kTa = [K_a | 0], kTb = [0 | K_b].
                # The score matmuls then run as full (128,128)-tile matmuls
                # like every other matmul in the kernel — no PE tile-config
                # switch (~100ns each) around the score pairs.
                kTa = kp.tile([128, NQC, CH], bf16, name="kTa", tag="ka")
                kTb = kp.tile([128, NQC, CH], bf16, name="kTb", tag="kb")
                vb = vp.tile([128, KT, VW], bf16, name="vb")
                # one full PSUM bank per batch; full-width [128,512] chains
                # (amortized LDWEIGHTS), split into 2 emission quanta so the
                # drain has ~2 attention slots before the bank is reused.
                par = pa.tile([128, CH], f32, name="ps_acc")
                quanta = []

                def q_ones():
                    nc.vector.memset(vb[:, :, D:D + 1], 1.0)
                    nc.vector.memset(vb[:, :, VW - 1:VW], 1.0)
                    nc.vector.memset(kTa[D:128, :, :], 0.0)
                    nc.vector.memset(kTb[0:D, :, :], 0.0)
                quanta.append(q_ones)

                for t in range(NQC):
                    st = {}

                    def q_dma(b=b, t=t, st=st, lo=0):
                        if lo == 0:
                            st["xts"] = [None] * KE
                        for k in range(lo, lo + KE // 2):
                            xt = xa.tile([128, CH], bf16, name="xt")
                            nc.sync.dma_start(
                                xt[:],
                                xT[k * 128:(k + 1) * 128,
                                   b * S + t * CH:b * S + (t + 1) * CH])
                            st["xts"][k] = xt
                        if b == 0 and t == 0 and lo == 0:
                            load_late_weights()
                    quanta.append(lambda b=b, t=t, st=st: q_dma(b, t, st, 0))
                    quanta.append(
                        lambda b=b, t=t, st=st: q_dma(b, t, st, KE // 2))

                    vt = va.tile([128, CH], bf16, name="vt")
                    for which, w_sb, b_name, dst in (
                            ("q", wq_sb, "bq", (qT[:, t, :],)),
                            ("k", wk_sb, "bk", (kTa[0:D, t, :],
                                                kTb[D:128, t, :])),
                            ("v", wv_sb, "bv", (vt[:],))):
                        # two half-chunk accumulation chains; each half
                        # drains (DVE cast) while the other runs, and the
                        # pa ring (2 half-bank slots) lets the next
                        # chain's start wait only on the older drain.
                        def q_chain(st=st, w_sb=w_sb, b_name=b_name,
                                    dst=dst, part=0):
                            ks = range(KE // 2) if part == 0 else \
                                range(KE // 2, KE)
                            for k in ks:
                                last = (k == KE - 1) and not with_qkv_bias
                                nc.tensor.matmul(par[:], w_sb[:, k, :],
                                                 st["xts"][k][:],
                                                 start=(k == 0), stop=last)
                            if part == 1:
                                if with_qkv_bias:
                                    bias_sb = {"bq": bq_sb, "bk": bk_sb,
                                               "bv": bv_sb}[b_name]
                                    nc.tensor.matmul(par[:], bias_sb[:],
                                                     ones_sb[:],
                                                     start=False, stop=True)
                                if len(dst) == 1:
                                    nc.vector.tensor_copy(dst[0], par[:])
                                else:
                                    nc.vector.tensor_copy(dst[0],
                                                          par[0:D, :])
                                    nc.vector.tensor_copy(dst[1],
                                                          par[D:128, :])
                        quanta.append(lambda f=q_chain: f(part=0))
                        quanta.append(lambda f=q_chain: f(part=1))

                    def q_tr(t=t, vt=vt, lo=0):
                        # PE tile-transpose of V into token-major key
                        # blocks (bf16 bitcast into half of the pa bank).
                        tr = par[:, lo * 128:(lo + 2) * 128].bitcast(
                            bf16).rearrange("p (a c) -> p a c", a=4)
                        for j in range(lo, lo + 2):
                            g = t * (CH // 128) + j
                            nc.tensor.transpose(
                                tr[:, j - lo, :],
                                vt[:, j * 128:(j + 1) * 128], id_sb[:])
                            nc.vector.tensor_copy(
                                vb[:, g, 0:D], tr[:, j - lo, 0:D])
                            nc.vector.tensor_copy(
                                vb[:, g, D + 1:2 * D + 1],
                                tr[:, j - lo, D:2 * D])
                    quanta.append(lambda t=t, vt=vt: q_tr(t, vt, 0))
                    quanta.append(lambda t=t, vt=vt: q_tr(t, vt, 2))
                return qT, (kTa, kTb), vb, quanta

            def phase_c_quanta(b, o_nm, m, pcr, tail=False):
                """One m-tile (128 tokens) of y = o_norm.T @ wo, as 2
                quanta of one full-width matmul + cast (+ y DMA); popping
                them >=1 slot apart hides the cast latency on the single
                phase-C PSUM bank.  In the tail (last batch, exps done)
                half the casts go to the then-idle scalar engine."""
                st = {}

                def q_pc(n):
                    mt = slice(m * 128, (m + 1) * 128)
                    ysl = slice(b * S + m * 128, b * S + (m + 1) * 128)
                    nch = slice(n * CH, (n + 1) * CH)
                    if n == 0:
                        st["y_sb"] = yc.tile([128, E], bf16, name="y_sb")
                    nc.tensor.matmul(pcr[:], o_nm[:, mt], wo_sb[:, nch])
                    if tail and n == 0:
                        nc.scalar.copy(st["y_sb"][:, nch], pcr[:])
                    else:
                        nc.vector.tensor_copy(st["y_sb"][:, nch], pcr[:])
                    nc.sync.dma_start(y[ysl, nch], st["y_sb"][:, nch])
                return [lambda: q_pc(0), lambda: q_pc(1)]

            def normalize_qc(qc, oT, o_nm, o_a, o_b):
                """o_norm[:, qc chunk] = oT * (1/c).  The denominator rows
                (partition 64 of each attnV psum tile) are reciprocal'd
                straight out of PSUM on the DVE, partition-broadcast on
                GPSIMD, then one DVE multiply per head."""
                crow = rcp.tile([1, 2 * CH], f32, name="crow")
                nc.vector.tensor_copy(crow[:, 0:CH], o_a[D:D + 1, :])
                nc.vector.tensor_copy(crow[:, CH:2 * CH], o_b[D:D + 1, :])
                rrow = rcp.tile([1, 2 * CH], f32, name="rrow")
                nc.vector.reciprocal_approx_fast(rrow[:], crow[:])
                rcb = rcp.tile([128, 2 * CH], f32, name="rcb")
                nc.gpsimd.partition_broadcast(rcb[:], rrow[:])
                span = slice(qc * CH, (qc + 1) * CH)
                nc.vector.tensor_tensor(o_nm[0:D, span], oT[0:D, span],
                                        rcb[0:D, 0:CH], op=MULT)
                nc.vector.tensor_tensor(o_nm[D:128, span], oT[D:128, span],
                                        rcb[D:128, CH:2 * CH], op=MULT)

            bg = deque()         # background emission quanta, FIFO
            pc_stash = []        # phase-C quanta held back one qc

            def pump(n=1):
                for _ in range(n):
                    if bg:
                        bg.popleft()()

            def attention(b, qT, kT, vb, last=False):
                """Attention for batch b.  Per kt-slot: score pair (row-
                tiled, concurrent), attnV pair (kt-2), exp(kt), then pump
                one background quantum so the in-order PE queue always has
                ~1.1us of work per ~1.0us exp."""
                oT = otp.tile([128, S], bf16, name="oT")
                o_nm = onp.tile([128, S], bf16, name="o_nm")
                pcr = pc.tile([128, CH], f32, name="ps_y")
                if last:
                    # no proj left in the last batch: borrow its PSUM bank
                    # so the tail's phase-C matmuls don't serialize on one
                    # accumulator's drain.
                    pcr2 = pa.tile([128, CH], f32, name="ps_acc")
                qv = qT[:].rearrange("p a c -> p (a c)")
                kva = kT[0][:].rearrange("p a c -> p (a c)")
                kvb = kT[1][:].rearrange("p a c -> p (a c)")
                NG = KT // 2
                for qc in range(NQC):
                    cols = slice(qc * CH, (qc + 1) * CH)
                    o_a = po.tile([D + 1, CH], f32, name="o_a")
                    o_b = po.tile([D + 1, CH], f32, name="o_b")
                    es = []
                    # 2-kt groups: 4 same-config attnV matmuls, then 2
                    # same-config row-tiled score pairs — each switch of
                    # the PE tile geometry costs ~100ns, so batching
                    # same-shape matmuls halves the reconfiguration tax.
                    for g in range(NG + 2):
                        if g >= 2:
                            for pk in (2 * (g - 2), 2 * (g - 2) + 1):
                                nc.tensor.matmul(o_a[:], vb[:, pk, 0:D + 1],
                                                 es[pk][:, 0, :],
                                                 start=(pk == 0),
                                                 stop=(pk == KT - 1))
                                nc.tensor.matmul(o_b[:], vb[:, pk, D + 1:VW],
                                                 es[pk][:, 1, :],
                                                 start=(pk == 0),
                                                 stop=(pk == KT - 1))
                        if g >= 2:
                            # pairs go AFTER the pumped background work,
                            # so by the time the PE reaches them both
                            # s_ab ring tiles are free (their exps
                            # finished a group ago).  In the first two
                            # groups of a qc (no attnV yet) the pairs go
                            # FIRST instead: the previous qc's exps are
                            # long done, and this keeps the scalar exp
                            # chain saturated across the qc boundary.
                            pump(2)
                        if g < NG:
                            sabs = []
                            for kt in (2 * g, 2 * g + 1):
                                kcols = slice(kt * 128, kt * 128 + 128)
                                s_ab = pbs.tile([128, 2, CH], f32,
                                                name="s_ab")
                                nc.tensor.matmul(s_ab[:, 0, :],
                                                 kva[:, kcols],
                                                 qv[:, cols])
                                nc.tensor.matmul(s_ab[:, 1, :],
                                                 kvb[:, kcols],
                                                 qv[:, cols])
                                sabs.append(s_ab)
                            for s_ab in sabs:
                                e_ab = eb.tile([128, 2, CH], bf16,
                                               name="e_ab")
                                nc.scalar.activation(e_ab[:], s_ab[:], EXP,
                                                     scale=0.125)
                                es.append(e_ab)
                        if g < 2:
                            pump(2)
                    nc.vector.tensor_copy(oT[0:D, cols], o_a[0:D, :])
                    nc.vector.tensor_copy(oT[D:2 * D, cols], o_b[0:D, :])
                    normalize_qc(qc, oT, o_nm, o_a, o_b)
                    # phase-C for this qc joins bg only at the NEXT qc
                    # boundary, so its matmuls never head-of-line-block the
                    # PE queue on the in-flight normalize chain.
                    bg.extend(pc_stash)
                    pc_stash.clear()
                    for m in range(4 * qc, 4 * qc + 4):
                        pc_stash.extend(phase_c_quanta(
                            b, o_nm, m, pcr2 if (last and m % 2) else pcr,
                            tail=(last and qc == NQC - 1)))
                return o_nm

            # --- pipeline over batches ---
            state = proj_quanta(0)
            q0 = state[3]
            for q in q0[:3]:     # t=0 x-loads issue before the warm-up
                q()
            emit_warmup()
            for q in q0[3:]:     # rest of batch 0's proj (ramp)
                q()
            for b in range(B):
                if b + 1 < B:
                    nxt = proj_quanta(b + 1)
                    bg.extend(nxt[3])
                attention(b, *state[:3], last=(b == B - 1))
                if b + 1 < B:
                    state = nxt
            bg.extend(pc_stash)
            pc_stash.clear()
            # filler matmuls ride out the last normalize chain's latency so
            # the HAM governor doesn't down-clock the PE before the final
            # phase-C flush.
            fill = pbs.tile([128, 2, CH], f32, name="s_ab")
            for _ in range(58):
                nc.tensor.matmul(fill[:, 0, 0:256], warm[:, 0:128],
                                 warm[:, 0:256], start=True, stop=True)
            while bg:
                bg.popleft()()

    nc.compile()
    return nc


def kernel(x, w_qkv, b_qkv, w_out, b_out):
    _install_ntff_hook()
    import jax.numpy as jnp

    x = np.ascontiguousarray(np.asarray(x, dtype=np.float32))
    w_qkv = np.asarray(w_qkv, dtype=np.float32)
    b_qkv = np.asarray(b_qkv, dtype=np.float32)
    w_out = np.asarray(w_out, dtype=np.float32)
    b_out = np.asarray(b_out, dtype=np.float32)

    with_bias = bool(np.any(b_qkv))
    key = ("mha", with_bias)
    if key not in _CACHE:
        _CACHE[key] = _build(with_bias)
    nc = _CACHE[key]

    def tobf(a):
        return np.asarray(jnp.asarray(a).astype(jnp.bfloat16))

    xT = tobf(np.ascontiguousarray(x.reshape(TOK, E).T))  # [E, TOK]
    ident = tobf(np.eye(128, dtype=np.float32))

    in_maps = []
    for c in range(NCORE):
        h0 = c * HPC
        qcols = slice(h0 * D, (h0 + HPC) * D)          # 128 q columns
        in_map = {
            "xT": xT,
            "wq": tobf(w_qkv[:, qcols]),
            "wk": tobf(w_qkv[:, E + h0 * D:E + (h0 + HPC) * D]),
            "wv": tobf(w_qkv[:, 2 * E + h0 * D:2 * E + (h0 + HPC) * D]),
            "wo": tobf(w_out[c * 128:(c + 1) * 128, :]),
            "ident": ident,
        }
        if with_bias:
            in_map["bq"] = tobf(b_qkv[qcols][None, :])
            in_map["bk"] = tobf(b_qkv[E + h0 * D:E + (h0 + HPC) * D][None, :])
            in_map["bv"] = tobf(
                b_qkv[2 * E + h0 * D:2 * E + (h0 + HPC) * D][None, :])
        in_maps.append(in_map)

    from concourse.bass_utils import run_bass_kernel_spmd

    trace = bool(globals().get("_TRACE"))
    res = run_bass_kernel_spmd(
        nc, in_maps, core_ids=list(range(NCORE)), trace=trace,
        **({"tmpdir": "/tmp/mha_trace"} if trace else {}))
    globals()["LAST_RES"] = res
    out = np.zeros((TOK, E), dtype=np.float64)
    for r in res.results:
        out += r["y"].astype(np.float64)
    out += b_out.astype(np.float64)
    return out.astype(np.float32).reshape(B, S, E)
